# revision 19
# baseline (speedup 1.0000x reference)
"""DepthToSpace (cell=4, 4 split groups) Trainium2 Bass kernel.

Full input x: [8, 64, 256, 256] f32 -> output [8, 4, 1024, 1024] f32.
out[b, s, 4h+r, 4w+c] = x[b, 16s + 4r + c, h, w]

Sharding: data parallel over batch — core b handles x[b] (16.8 MB in/out).

Per-core plan (pure data movement, memory-bound). HW facts measured with
probe kernels on these parts:
  * A DMA instruction's descriptors go to the 16 SDMA engines in
    consecutive ceil(outer_dim/16) blocks of the DRAM-side outer AP dim.
  * SBUF AXI ports are wired to fixed partitions
    (port(P) = ((P%32)//4)*2 + P//64); an instruction only gets the
    bandwidth of the ports its partitions touch, so every instruction
    here spans partitions covering all 16 ports.
  * Per-engine ~29 GB/s wire + ~15 ns/descriptor; aggregate ~410 GB/s
    at 4 KB descs, ~434+ at 32 KB.

Layout: partition p = 4*hb + s holds input rows h in [8*hb, 8*hb+8) of
all 16 channels of split group s (partitions s::4 span all 16 ports).
  load : per (p, ch) 4 KB runs; per (group, h-half) one 512-desc DMA
         (outer dim hb=32 -> all 16 engines, 2 hb each).
  store: y[s] rows [32hb+4j, 32hb+4j+4) are one contiguous 16 KB run
         per partition; chunk j is a single 128-desc DMA (all engines).
The shuffle Y[r, w, c] = X[4r+c, h8=j, w] stays INTRA-partition on
DVE+ACT (5:3 unit split; iterate [p, c, w] so the source reads
contiguous 1 KB runs — measured 634 ns vs 781 ns per 64 K-elem unit),
overlapped under the DMA. NYB=4 Y buffers give ~3 store-times of
recycle slack so store completion latency stays off the shuffle path.

Single HWDGE ring (Sync): loads enqueue first, stores FIFO behind them.
"""

import sys

sys.path.insert(0, "/opt/trn_rl_repo")

import numpy as np

import concourse.bass as bass
import concourse.mybir as mybir
from concourse.bass_utils import run_bass_kernel_spmd

B, C, H, W = 8, 64, 256, 256
S = 4
CELL = 4  # sqrt(C // S)
CPG = C // S  # channels per group = 16
P = 128  # SBUF partitions
HB = 32  # h-blocks per group (partition p = 4*hb + s)
N_CORES = 8

NYB = 4  # Y buffers (16 KB each)
NCHUNK = 8  # store chunks (one h8 row each)

# Shuffle work units per chunk: (r, whalf) -> 8 units, DVE:ACT = 5:3.
ALL_UNITS = [(r, wh) for r in range(CELL) for wh in range(2)]
DVE_UNITS = ALL_UNITS[:5]
ACT_UNITS = ALL_UNITS[5:]
WH = W // 2


def build_program():
    nc = bass.Bass()
    x = nc.declare_dram_parameter("x", [C, H, W], mybir.dt.float32, isOutput=False)
    y = nc.declare_dram_parameter(
        "y", [S, H * CELL, W * CELL], mybir.dt.float32, isOutput=True
    )

    from contextlib import ExitStack

    with ExitStack() as ctx:
        # X[p][ch, hj, h4, w]: 16*2*4*256 f32 = 128 KB per partition
        Xt = ctx.enter_context(
            nc.sbuf_tensor("X", [P, CPG, 2, 4, W], mybir.dt.float32)
        )
        # Y[b][p][r, w, c]: 4*256*4 f32 = 16 KB
        Yt = [
            ctx.enter_context(
                nc.sbuf_tensor(f"Y{i}", [P, CELL, W, CELL], mybir.dt.float32)
            )
            for i in range(NYB)
        ]
        inl = [ctx.enter_context(nc.semaphore(f"inl{i}")) for i in range(2)]
        outs = [ctx.enter_context(nc.semaphore(f"outs{i}")) for i in range(NYB)]
        shuf_v = ctx.enter_context(nc.semaphore("shuf_v"))
        shuf_a = ctx.enter_context(nc.semaphore("shuf_a"))
        block = ctx.enter_context(nc.Block(no_gpsimd_drain=True))

        # x viewed as [s, hb, ch, hj, h4, w]
        xv = x.rearrange(
            "(s ch) (hb hj h4) w -> s hb ch hj h4 w", s=S, hb=HB, hj=2, h4=4
        )

        def store_ap(j):
            # y[s] rows [32hb + 4j, 32hb + 4j + 4) <- Yt[j%NYB]:
            # one 16 KB contiguous run per partition, outer dim hb=32.
            return y.rearrange(
                "s (hb j r) (w c) -> hb s j r w c",
                hb=HB,
                j=NCHUNK,
                r=CELL,
                c=CELL,
            )[:, :, j]

        # X as [p, r, c, hj, h4, w] for the shuffle (ch = 4r + c)
        xr_fn = lambda: Xt[:].rearrange(
            "p (r c) hj h4 w -> p r c hj h4 w", r=CELL, c=CELL
        )

        def copy_aps(j, r, wh):
            # chunk j covers h8 = j; h8 = 4*hj + h4.
            # Iterate [p, c, w]: src reads contiguous along w (1 KB runs),
            # dst pays the 16 B-strided writes.
            hj, h4 = divmod(j, 4)
            wlo, whi = wh * WH, (wh + 1) * WH
            src = xr_fn()[:, r, :, hj, h4, wlo:whi]  # [p, c, w]
            dst = Yt[j % NYB][:, r, wlo:whi]  # [p, w, c]
            dst = dst.transpose([0, 2, 1])  # [p, c, w]
            return src, dst

        n_dve = len(DVE_UNITS)
        n_act = len(ACT_UNITS)

        @block.sync
        def _(sync):
            for hj in range(2):
                for s in range(S):
                    sync.dma_start(
                        out=Xt[s::S, :, hj],
                        in_=xv[s, :, :, hj],
                    ).then_inc(inl[hj], 16)
            for j in range(NCHUNK):
                sync.wait_ge(shuf_v, n_dve * (j + 1))
                sync.wait_ge(shuf_a, n_act * (j + 1))
                sync.dma_start(out=store_ap(j), in_=Yt[j % NYB][:]).then_inc(
                    outs[j % NYB], 16
                )
            for b in range(NYB):
                sync.wait_ge(outs[b], 16 * (NCHUNK // NYB))

        @block.vector
        def _(vector):
            for j in range(NCHUNK):
                vector.wait_ge(inl[j // 4], 64)
                if j >= NYB:
                    vector.wait_ge(outs[j % NYB], 16 * (j // NYB))
                for r, wh in DVE_UNITS:
                    src, dst = copy_aps(j, r, wh)
                    vector.tensor_copy(out=dst, in_=src).then_inc(shuf_v, 1)

        @block.scalar
        def _(scalar):
            for j in range(NCHUNK):
                scalar.wait_ge(inl[j // 4], 64)
                if j >= NYB:
                    scalar.wait_ge(outs[j % NYB], 16 * (j // NYB))
                for r, wh in ACT_UNITS:
                    src, dst = copy_aps(j, r, wh)
                    scalar.copy(out=dst, in_=src).then_inc(shuf_a, 1)

    return nc


def run_sharded(x: np.ndarray, trace: bool = False):
    """Shard x over batch across 8 cores, run, gather. Returns (out, results)."""
    assert x.shape == (B, C, H, W), x.shape
    nc = build_program()
    in_maps = [{"x": np.ascontiguousarray(x[b])} for b in range(N_CORES)]
    res = run_bass_kernel_spmd(nc, in_maps, list(range(N_CORES)), trace=trace)
    out = np.stack([res.results[b]["y"] for b in range(N_CORES)], axis=0)
    return out.astype(x.dtype, copy=False), res


def kernel(**inputs: np.ndarray) -> np.ndarray:
    x = np.asarray(inputs["x"], dtype=np.float32)
    out, _ = run_sharded(x, trace=False)
    return out


# revision 20
# speedup vs baseline: 1.0154x; 1.0154x over previous
"""DepthToSpace (cell=4, 4 split groups) Trainium2 Bass kernel.

Full input x: [8, 64, 256, 256] f32 -> output [8, 4, 1024, 1024] f32.
out[b, s, 4h+r, 4w+c] = x[b, 16s + 4r + c, h, w]

Sharding: data parallel over batch — core b handles x[b] (16.8 MB in/out).

Per-core plan (pure data movement, memory-bound). HW facts measured with
probe kernels on these parts:
  * A DMA instruction's descriptors go to the 16 SDMA engines in
    consecutive ceil(outer_dim/16) blocks of the DRAM-side outer AP dim.
  * SBUF AXI ports are wired to fixed partitions
    (port(P) = ((P%32)//4)*2 + P//64); an instruction only gets the
    bandwidth of the ports its partitions touch, so every instruction
    here spans partitions covering all 16 ports.
  * Per-engine ~29 GB/s wire + ~15 ns/descriptor; aggregate ~410 GB/s
    at 4 KB descs, ~434+ at 32 KB.

Layout: partition p = 4*hb + s holds input rows h in [8*hb, 8*hb+8) of
all 16 channels of split group s (partitions s::4 span all 16 ports).
  load : per (p, ch) 4 KB runs; per (group, h-half) one 512-desc DMA
         (outer dim hb=32 -> all 16 engines, 2 hb each).
  store: chunk pairs share one Y tensor whose adjacent 16 KB slots are
         contiguous, so each pair is stored as one 128-desc DMA of 32 KB
         descriptors (rows [32hb+8jp, 32hb+8jp+8) per partition).
The shuffle Y[r, w, c] = X[4r+c, h8=j, w] stays INTRA-partition on
DVE+ACT (5:3 unit split; iterate [p, c, w] so the source reads
contiguous 1 KB runs — measured 634 ns vs 781 ns per 64 K-elem unit),
overlapped under the DMA. NYB=4 Y buffers give ~3 store-times of
recycle slack so store completion latency stays off the shuffle path.

Single HWDGE ring (Sync): loads enqueue first, stores FIFO behind them.
"""

import sys

sys.path.insert(0, "/opt/trn_rl_repo")

import numpy as np

import concourse.bass as bass
import concourse.mybir as mybir
from concourse.bass_utils import run_bass_kernel_spmd

B, C, H, W = 8, 64, 256, 256
S = 4
CELL = 4  # sqrt(C // S)
CPG = C // S  # channels per group = 16
P = 128  # SBUF partitions
HB = 32  # h-blocks per group (partition p = 4*hb + s)
N_CORES = 8

NYB = 4  # Y buffers (16 KB each)
NCHUNK = 8  # store chunks (one h8 row each)

# Shuffle work units per chunk: (r, whalf) -> 8 units, DVE:ACT = 5:3.
ALL_UNITS = [(r, wh) for r in range(CELL) for wh in range(2)]
DVE_UNITS = ALL_UNITS[:5]
ACT_UNITS = ALL_UNITS[5:]
WH = W // 2


def build_program():
    nc = bass.Bass()
    x = nc.declare_dram_parameter("x", [C, H, W], mybir.dt.float32, isOutput=False)
    y = nc.declare_dram_parameter(
        "y", [S, H * CELL, W * CELL], mybir.dt.float32, isOutput=True
    )

    from contextlib import ExitStack

    with ExitStack() as ctx:
        # X[p][ch, hj, h4, w]: 16*2*4*256 f32 = 128 KB per partition
        Xt = ctx.enter_context(
            nc.sbuf_tensor("X", [P, CPG, 2, 4, W], mybir.dt.float32)
        )
        # Y[p][slot, r, w, c]: one tensor, 4 slots x 16 KB. Adjacent slot
        # pairs (0,1) and (2,3) are contiguous 32 KB in SBUF, matching the
        # DRAM contiguity of chunk pairs -> 32 KB store descriptors.
        Yt = ctx.enter_context(
            nc.sbuf_tensor("Y", [P, NYB, CELL, W, CELL], mybir.dt.float32)
        )
        inl = [ctx.enter_context(nc.semaphore(f"inl{i}")) for i in range(2)]
        outs = [ctx.enter_context(nc.semaphore(f"outs{i}")) for i in range(2)]
        shuf_v = ctx.enter_context(nc.semaphore("shuf_v"))
        shuf_a = ctx.enter_context(nc.semaphore("shuf_a"))
        block = ctx.enter_context(nc.Block(no_gpsimd_drain=True))

        # x viewed as [s, hb, ch, hj, h4, w]
        xv = x.rearrange(
            "(s ch) (hb hj h4) w -> s hb ch hj h4 w", s=S, hb=HB, hj=2, h4=4
        )

        def store_ap(jp):
            # Pair jp covers chunks {2jp, 2jp+1} = y[s] rows
            # [32hb + 8jp, 32hb + 8jp + 8): one 32 KB contiguous run per
            # partition (outer dim hb=32 -> all 16 engines).
            return y.rearrange(
                "s (hb jp h2 r) (w c) -> hb s jp h2 r w c",
                hb=HB,
                jp=NCHUNK // 2,
                h2=2,
                r=CELL,
                c=CELL,
            )[:, :, jp]

        # X as [p, r, c, hj, h4, w] for the shuffle (ch = 4r + c)
        xr_fn = lambda: Xt[:].rearrange(
            "p (r c) hj h4 w -> p r c hj h4 w", r=CELL, c=CELL
        )

        def copy_aps(j, r, wh):
            # chunk j covers h8 = j; h8 = 4*hj + h4.
            # Iterate [p, c, w]: src reads contiguous along w (1 KB runs),
            # dst pays the 16 B-strided writes.
            hj, h4 = divmod(j, 4)
            wlo, whi = wh * WH, (wh + 1) * WH
            src = xr_fn()[:, r, :, hj, h4, wlo:whi]  # [p, c, w]
            dst = Yt[:, j % NYB, r, wlo:whi]  # [p, w, c]
            dst = dst.transpose([0, 2, 1])  # [p, c, w]
            return src, dst

        n_dve = len(DVE_UNITS)
        n_act = len(ACT_UNITS)

        @block.sync
        def _(sync):
            for hj in range(2):
                for s in range(S):
                    sync.dma_start(
                        out=Xt[s::S, :, hj],
                        in_=xv[s, :, :, hj],
                    ).then_inc(inl[hj], 16)
            for jp in range(NCHUNK // 2):
                sync.wait_ge(shuf_v, n_dve * (2 * jp + 2))
                sync.wait_ge(shuf_a, n_act * (2 * jp + 2))
                slot = (2 * jp) % NYB
                sync.dma_start(
                    out=store_ap(jp), in_=Yt[:, slot : slot + 2]
                ).then_inc(outs[jp % 2], 16)
            sync.wait_ge(outs[0], 32)
            sync.wait_ge(outs[1], 32)

        @block.vector
        def _(vector):
            for j in range(NCHUNK):
                vector.wait_ge(inl[j // 4], 64)
                if j >= NYB:
                    vector.wait_ge(outs[((j - NYB) // 2) % 2], 16)
                for r, wh in DVE_UNITS:
                    src, dst = copy_aps(j, r, wh)
                    vector.tensor_copy(out=dst, in_=src).then_inc(shuf_v, 1)

        @block.scalar
        def _(scalar):
            for j in range(NCHUNK):
                scalar.wait_ge(inl[j // 4], 64)
                if j >= NYB:
                    scalar.wait_ge(outs[((j - NYB) // 2) % 2], 16)
                for r, wh in ACT_UNITS:
                    src, dst = copy_aps(j, r, wh)
                    scalar.copy(out=dst, in_=src).then_inc(shuf_a, 1)

    return nc


def run_sharded(x: np.ndarray, trace: bool = False):
    """Shard x over batch across 8 cores, run, gather. Returns (out, results)."""
    assert x.shape == (B, C, H, W), x.shape
    nc = build_program()
    in_maps = [{"x": np.ascontiguousarray(x[b])} for b in range(N_CORES)]
    res = run_bass_kernel_spmd(nc, in_maps, list(range(N_CORES)), trace=trace)
    out = np.stack([res.results[b]["y"] for b in range(N_CORES)], axis=0)
    return out.astype(x.dtype, copy=False), res


def kernel(**inputs: np.ndarray) -> np.ndarray:
    x = np.asarray(inputs["x"], dtype=np.float32)
    out, _ = run_sharded(x, trace=False)
    return out
